# revision 15
# baseline (speedup 1.0000x reference)
"""Trainium2 Bass kernel for nn_BaselineMNISTClassifier (vq_codebook).

reference:
    x = samples - 0.5                        # [B, F]
    hv = einsum('bf,df->bd', x, bhv)         # [B, D]
    e = (hv > 0)                             # binary
    ham[b, c] = sum_d |e - centroids[c, d]|  # [B, C]
    return -ham

Identity used on device: with s = sign(hv) in {-1, +1} and
cmod = 1 - 2c in {-1, +1}:  |e - c| = s * cmod / 2 + 1/2, so
    ham[b, c] = (sum_d s[b, d] * cmod[c, d]) / 2 + D/2
which turns the broadcast Hamming into a second (tiny) matmul.

Sharding: D axis (10000) split across 8 cores, 1250 (padded to 1280) per
core. Each core computes full-batch partial hamming [C, B]; partials sum
on the host (padded dims contribute exactly 0: centroid pad value 0.5
makes cmod = 0 there, and sign(0) = 0 besides).

Encode matmul runs in float32r (~tf32 precision, full PE rate at N=512).
Both operands are host-transposed so the contraction dim F sits on SBUF
partitions. Second matmul runs in bf16 (s, cmod exact in bf16).

Perf structure (per core):
  - warmup matmuls release the PE HAM clock gate while inputs stream;
    extra dummy fills inside the first (DMA-bound) d-tile group keep
    the clock warm through the ramp
  - x DMA triggers alternate between the SP and Scalar queues (each
    trigger costs ~0.6 us of issue time); w/centroid/output triggers go
    to GpSimd
  - four b-groups of 2 blocks: the first group's encode only needs
    3.5 MB of x before it can run at full rate
  - both hamming accumulators of a group live in ONE PSUM bank at
    partition offsets 0/32, col-tiled (tile_position) so they overlap
    on the PE array; 6 PSUM banks feed the encode accumulation
  - binarize (Sign) on the Scalar engine, centering on DVE
  - hamming matmuls emitted one d-tile late so PE never waits on the
    binarize; epilogue on Scalar, output DMA per b-block immediately
"""

import sys

sys.path.insert(0, "/opt/trn_rl_repo")

import numpy as np

import concourse.bacc as bacc
import concourse.bass as bass
import concourse.mybir as mybir
import concourse.tile as tile
from concourse.bass_utils import run_bass_kernel_spmd

B = 4096
F = 784
D = 10000
C = 10
NCORES = 8
DREAL = D // NCORES          # 1250 real dims per core
DP = 1280                    # padded to 10 d-tiles of 128
ND = DP // 128               # 10
NB = B // 512                # 8 b-blocks of 512
FT = [(i * 128, min(128, F - i * 128)) for i in range((F + 127) // 128)]
NF = len(FT)                 # 7 (6x128 + 16)
NWARM = 60                   # PE warmup matmuls
NFILL = 5                    # dummy matmuls per fi-step of the first group

F32 = mybir.dt.float32
F32R = mybir.dt.float32r
BF16 = mybir.dt.bfloat16
OP = mybir.AluOpType
AF = mybir.ActivationFunctionType

_NC_CACHE = {}


def _build_nc():
    if "nc" in _NC_CACHE:
        return _NC_CACHE["nc"]
    nc = bacc.Bacc("TRN2", debug=False, target_bir_lowering=False)
    xT = nc.dram_tensor("xT", [F, B], F32R, kind="ExternalInput")
    wT = nc.dram_tensor("wT", [F, DP], F32R, kind="ExternalInput")
    cT = nc.dram_tensor("cT", [DP, C], F32, kind="ExternalInput")
    out = nc.dram_tensor("out", [C, B], F32, kind="ExternalOutput")

    with tile.TileContext(nc) as tc:
        with (
            tc.tile_pool(name="dum", bufs=2) as dumpool,
            tc.tile_pool(name="xp", bufs=NB // 2 * NF) as xpool,
            tc.tile_pool(name="wp", bufs=(ND + 1) // 2 * NF) as wpool,
            tc.tile_pool(name="cp", bufs=1) as cpool,
            tc.tile_pool(name="cmp", bufs=1) as cmpool,
            tc.tile_pool(name="ep", bufs=6) as epool,
            tc.tile_pool(name="op", bufs=4) as opool,
            tc.tile_pool(name="pse", bufs=7, space="PSUM") as psepool,
            tc.tile_pool(name="ps2", bufs=1, space="PSUM") as ps2pool,
        ):
            # --- PE warmup: release the HAM clock gate while inputs load.
            wdum = dumpool.tile([128, 128], BF16)
            nc.gpsimd.memset(wdum[:], 1.0)
            xdum = dumpool.tile([128, 512], BF16)
            nc.gpsimd.memset(xdum[:], 1.0)
            psdum = psepool.tile([128, 512], F32, name="psdum", tag="pse")
            for i in range(NWARM):
                nc.tensor.matmul(psdum[:], wdum[:], xdum[:],
                                 start=(i == 0), stop=(i == NWARM - 1))

            # --- centroid prep: one DMA for all 10 d-tiles, then
            # cmod = 1 - 2c (bf16). Pad rows are 0.5 -> cmod = 0.
            ct = cpool.tile([128, ND * C], F32)
            nc.gpsimd.dma_start(
                ct[:].rearrange("p (a c) -> p a c", c=C),
                cT.ap().rearrange("(a p) c -> p a c", p=128))
            cmod = cmpool.tile([128, ND * C], BF16)
            nc.scalar.activation(cmod[:], ct[:], AF.Copy, bias=1.0,
                                 scale=-2.0)
            cmods = [cmod[:, di * C:(di + 1) * C] for di in range(ND)]

            # --- input loads; tiles single-assignment (loaded once, no
            # slot reuse) so input DMAs never carry data waits. x tiles
            # span two b-blocks, w tiles two d-tiles.
            xts = {}
            wts = {}

            def load_x(bp, fi):   # bp = b-block pair index (0..3)
                f0, fl = FT[fi]
                xt = xpool.tile([fl, 1024], F32R, name=f"xt_{bp}_{fi}",
                                tag="xt")
                nc.sync.dma_start(
                    xt[:], xT[f0:f0 + fl, bp * 1024:(bp + 1) * 1024])
                # center (x - 0.5) in place on DVE
                nc.vector.tensor_scalar_add(xt[:], xt[:], -0.5)
                xts[bp, fi] = xt

            def load_w(dp, fi):   # dp = d-tile pair index (0..4)
                f0, fl = FT[fi]
                wid = min(256, DP - dp * 256)
                wt = wpool.tile([fl, wid], F32R, name=f"wt_{dp}_{fi}",
                                tag="wt")
                nc.gpsimd.dma_start(
                    wt[:], wT[f0:f0 + fl, dp * 256:dp * 256 + wid])
                wts[dp, fi] = wt

            for i in range(5):
                for fi in range(NF):
                    if i < 4:
                        load_x(i, fi)
                    load_w(i, fi)

            def xop(bb, fi):
                return xts[bb // 2, fi][:, (bb % 2) * 512:(bb % 2 + 1) * 512]

            def wop(di, fi):
                return wts[di // 2, fi][:, (di % 2) * 128:(di % 2 + 1) * 128]

            # --- main compute: two b-groups of 4 blocks.
            for bg in range(2):
                bbs = list(range(bg * 4, bg * 4 + 4))
                ps2 = ps2pool.tile([128, 512], F32, name=f"ps2_{bg}",
                                   tag="ps2")
                psum2 = {bb: ps2[32 * (bb % 4):32 * (bb % 4) + C, :]
                         for bb in bbs}
                pending = []
                for di in range(ND):
                    pses = {}
                    for bb in bbs:
                        pses[bb] = psepool.tile([128, 512], F32,
                                                name=f"pse_{di % 2}_{bb}",
                                                tag="pse")
                    for fi in range(NF):
                        for bb in bbs:
                            nc.tensor.matmul(pses[bb][:], wop(di, fi),
                                             xop(bb, fi),
                                             start=(fi == 0),
                                             stop=(fi == NF - 1))
                    ets = {}
                    for bb in bbs:
                        # e' = (hv > 0) - 0.5 in {-1/2, +1/2}
                        et = epool.tile([128, 512], BF16,
                                        name=f"et_{di % 2}_{bb}", tag="et")
                        nc.vector.tensor_scalar(et[:], pses[bb][:], 0.0,
                                                0.5, op0=OP.is_gt,
                                                op1=OP.subtract)
                        ets[bb] = et
                    for pdi, pbb, pet in pending:
                        nc.tensor.matmul(psum2[pbb], cmods[pdi],
                                         pet[:], start=(pdi == 0),
                                         stop=(pdi == ND - 1),
                                         tile_position=(0, 32 * (pbb % 4)))
                    pending = [(di, bb, ets[bb]) for bb in bbs]
                for pdi, pbb, pet in pending:
                    nc.tensor.matmul(psum2[pbb], cmods[pdi], pet[:],
                                     start=(pdi == 0), stop=(pdi == ND - 1),
                                     tile_position=(0, 32 * (pbb % 4)))
                    # out = -(psum2 + DREAL/2), on the Scalar engine
                    ot = opool.tile([C, 512], F32, name=f"ot_{pbb % 4}",
                                    tag="ot")
                    nc.scalar.activation(ot[:], psum2[pbb], AF.Copy,
                                         bias=-float(DREAL) / 2.0,
                                         scale=-1.0)
                    nc.gpsimd.dma_start(
                        out[:, pbb * 512:(pbb + 1) * 512], ot[:])
    nc.compile()
    _NC_CACHE["nc"] = nc
    return nc


def _prep_in_maps(samples, bhv_matrix, centroids):
    samples = np.ascontiguousarray(samples, dtype=np.float32)
    bhv_matrix = np.ascontiguousarray(bhv_matrix, dtype=np.float32)
    centroids = np.ascontiguousarray(centroids, dtype=np.float32)
    xT = np.ascontiguousarray(samples.T)  # [F, B]
    in_maps = []
    for k in range(NCORES):
        lo_, hi_ = k * DREAL, (k + 1) * DREAL
        wTk = np.zeros((F, DP), dtype=np.float32)
        wTk[:, :DREAL] = bhv_matrix[lo_:hi_, :].T
        cTk = np.full((DP, C), 0.5, dtype=np.float32)
        cTk[:DREAL, :] = centroids[:, lo_:hi_].T
        in_maps.append({"xT": xT, "wT": wTk, "cT": cTk})
    return in_maps


def _run(samples, bhv_matrix, centroids, **spmd_kwargs):
    nc = _build_nc()
    in_maps = _prep_in_maps(samples, bhv_matrix, centroids)
    res = run_bass_kernel_spmd(nc, in_maps, core_ids=list(range(NCORES)),
                               **spmd_kwargs)
    acc = np.zeros((C, B), dtype=np.float32)
    for r in res.results:
        acc += r["out"]
    return np.ascontiguousarray(acc.T), res


def kernel(samples, bhv_matrix, centroids):
    out, _ = _run(samples, bhv_matrix, centroids)
    return out


# revision 16
# speedup vs baseline: 1.0226x; 1.0226x over previous
"""Trainium2 Bass kernel for nn_BaselineMNISTClassifier (vq_codebook).

reference:
    x = samples - 0.5                        # [B, F]
    hv = einsum('bf,df->bd', x, bhv)         # [B, D]
    e = (hv > 0)                             # binary
    ham[b, c] = sum_d |e - centroids[c, d]|  # [B, C]
    return -ham

Identity used on device: with e' = (hv > 0) - 0.5 in {-1/2, +1/2} and
cmod = 1 - 2c in {-1, +1}:  |e - c| = e' * cmod + 1/2, so
    ham[b, c] = sum_d e'[b, d] * cmod[c, d] + D/2
which turns the broadcast Hamming into a second (tiny) matmul over the
same d-tiles.

Sharding: the D axis (10000) splits across 8 cores, 1250 (zero-padded
to 1280) per core. Every core sees the full batch and computes a
partial hamming [C, B]; the partials sum on the host (padded dims
contribute exactly 0: the centroid pad value 0.5 makes cmod = 0 there).

The encode matmul runs in float32r (~tf32 precision; streams one
column per cycle at N=512, measured 227 ns / matmul warm). Both
operands are host-transposed so the contraction dim F sits on SBUF
partitions; no on-device transposes anywhere. The hamming matmul runs
in bf16 (e', cmod are exact in bf16), so the device output is exact
integer arithmetic given the encode bits.

Perf structure (per core, measured ~181 us on hardware):
  - 60 warmup matmuls on dummy data release the PE HAM clock gate
    (1.2 -> 2.4 GHz) while the inputs stream in; the clock then stays
    warm for the whole kernel
  - input tiles are single-assignment (no slot reuse), so input DMAs
    never carry data-dependency waits; x triggers issue from SP, w and
    centroid/output triggers from GpSimd (each DMA trigger costs
    ~0.6 us of issue time on its engine)
  - fi-outer / bb-inner matmul order: 4 consecutive matmuls share the
    stationary weights, hiding the fused fp32r LDWEIGHTS
  - all four hamming accumulators of a b-group live in ONE PSUM bank
    at partition offsets 0/32/64/96 via col-tiled matmuls
    (tile_position), which frees 7 PSUM banks for the encode
    accumulation (deep multi-buffering, no start-of-group stalls)
  - hamming matmuls are emitted one d-tile late so the PE never waits
    on the DVE binarize; the epilogue alternates Scalar/DVE and each
    output block DMAs out as soon as its accumulation closes

Toolchain notes: built on bacc.Bacc (its compile() legalizes the
1-sync-wait-per-instruction hardware limit via event semaphores, which
raw Bass + TileContext does not); output DMAs go through nc.gpsimd
because SP DMA_DIRECT2D triggers only take a single wait.
"""

import sys

sys.path.insert(0, "/opt/trn_rl_repo")

import numpy as np

import concourse.bacc as bacc
import concourse.bass as bass
import concourse.mybir as mybir
import concourse.tile as tile
from concourse.bass_utils import run_bass_kernel_spmd

B = 4096
F = 784
D = 10000
C = 10
NCORES = 8
DREAL = D // NCORES          # 1250 real dims per core
DP = 1280                    # padded to 10 d-tiles of 128
ND = DP // 128               # 10
NB = B // 512                # 8 b-blocks of 512
FT = [(i * 128, min(128, F - i * 128)) for i in range((F + 127) // 128)]
NF = len(FT)                 # 7 (6x128 + 16)
NWARM = 60                   # PE warmup matmuls
NFILL = 5                    # dummy matmuls per fi-step of the first group

F32 = mybir.dt.float32
F32R = mybir.dt.float32r
BF16 = mybir.dt.bfloat16
OP = mybir.AluOpType
AF = mybir.ActivationFunctionType

_NC_CACHE = {}


def _build_nc():
    if "nc" in _NC_CACHE:
        return _NC_CACHE["nc"]
    nc = bacc.Bacc("TRN2", debug=False, target_bir_lowering=False)
    xT = nc.dram_tensor("xT", [F, B], F32R, kind="ExternalInput")
    wT = nc.dram_tensor("wT", [F, DP], F32R, kind="ExternalInput")
    cT = nc.dram_tensor("cT", [DP, C], F32, kind="ExternalInput")
    out = nc.dram_tensor("out", [C, B], F32, kind="ExternalOutput")

    with tile.TileContext(nc) as tc:
        with (
            tc.tile_pool(name="dum", bufs=2) as dumpool,
            tc.tile_pool(name="xp", bufs=NB // 2 * NF) as xpool,
            tc.tile_pool(name="wp", bufs=(ND + 1) // 2 * NF) as wpool,
            tc.tile_pool(name="cp", bufs=1) as cpool,
            tc.tile_pool(name="cmp", bufs=1) as cmpool,
            tc.tile_pool(name="ep", bufs=6) as epool,
            tc.tile_pool(name="op", bufs=4) as opool,
            tc.tile_pool(name="pse", bufs=7, space="PSUM") as psepool,
            tc.tile_pool(name="ps2", bufs=1, space="PSUM") as ps2pool,
        ):
            # --- PE warmup: release the HAM clock gate while inputs load.
            wdum = dumpool.tile([128, 128], BF16)
            nc.gpsimd.memset(wdum[:], 1.0)
            xdum = dumpool.tile([128, 512], BF16)
            nc.gpsimd.memset(xdum[:], 1.0)
            psdum = psepool.tile([128, 512], F32, name="psdum", tag="pse")
            for i in range(NWARM):
                nc.tensor.matmul(psdum[:], wdum[:], xdum[:],
                                 start=(i == 0), stop=(i == NWARM - 1))

            # --- centroid prep: one DMA for all 10 d-tiles, then
            # cmod = 1 - 2c (bf16). Pad rows are 0.5 -> cmod = 0.
            ct = cpool.tile([128, ND * C], F32)
            nc.gpsimd.dma_start(
                ct[:].rearrange("p (a c) -> p a c", c=C),
                cT.ap().rearrange("(a p) c -> p a c", p=128))
            cmod = cmpool.tile([128, ND * C], BF16)
            nc.scalar.activation(cmod[:], ct[:], AF.Copy, bias=1.0,
                                 scale=-2.0)
            cmods = [cmod[:, di * C:(di + 1) * C] for di in range(ND)]

            # --- input loads; tiles single-assignment (loaded once, no
            # slot reuse) so input DMAs never carry data waits. x tiles
            # span two b-blocks, w tiles two d-tiles.
            xts = {}
            wts = {}

            def load_x(bp, fi):   # bp = b-block pair index (0..3)
                f0, fl = FT[fi]
                xt = xpool.tile([fl, 1024], F32R, name=f"xt_{bp}_{fi}",
                                tag="xt")
                nc.sync.dma_start(
                    xt[:], xT[f0:f0 + fl, bp * 1024:(bp + 1) * 1024])
                # center (x - 0.5) in place on DVE
                nc.vector.tensor_scalar_add(xt[:], xt[:], -0.5)
                xts[bp, fi] = xt

            def load_w(dp, fi):   # dp = d-tile pair index (0..4)
                f0, fl = FT[fi]
                wid = min(256, DP - dp * 256)
                wt = wpool.tile([fl, wid], F32R, name=f"wt_{dp}_{fi}",
                                tag="wt")
                nc.gpsimd.dma_start(
                    wt[:], wT[f0:f0 + fl, dp * 256:dp * 256 + wid])
                wts[dp, fi] = wt

            for i in range(5):
                for fi in range(NF):
                    if i < 4:
                        load_x(i, fi)
                    load_w(i, fi)

            def xop(bb, fi):
                return xts[bb // 2, fi][:, (bb % 2) * 512:(bb % 2 + 1) * 512]

            def wop(di, fi):
                return wts[di // 2, fi][:, (di % 2) * 128:(di % 2 + 1) * 128]

            # --- main compute: two b-groups of 4 blocks.
            for bg in range(2):
                bbs = list(range(bg * 4, bg * 4 + 4))
                ps2 = ps2pool.tile([128, 512], F32, name=f"ps2_{bg}",
                                   tag="ps2")
                psum2 = {bb: ps2[32 * (bb % 4):32 * (bb % 4) + C, :]
                         for bb in bbs}
                pending = []
                for di in range(ND):
                    pses = {}
                    for bb in bbs:
                        pses[bb] = psepool.tile([128, 512], F32,
                                                name=f"pse_{di % 2}_{bb}",
                                                tag="pse")
                    for fi in range(NF):
                        for bb in bbs:
                            nc.tensor.matmul(pses[bb][:], wop(di, fi),
                                             xop(bb, fi),
                                             start=(fi == 0),
                                             stop=(fi == NF - 1))
                    ets = {}
                    for bb in bbs:
                        # e' = (hv > 0) - 0.5 in {-1/2, +1/2}
                        et = epool.tile([128, 512], BF16,
                                        name=f"et_{di % 2}_{bb}", tag="et")
                        nc.vector.tensor_scalar(et[:], pses[bb][:], 0.0,
                                                0.5, op0=OP.is_gt,
                                                op1=OP.subtract)
                        ets[bb] = et
                    for pdi, pbb, pet in pending:
                        nc.tensor.matmul(psum2[pbb], cmods[pdi],
                                         pet[:], start=(pdi == 0),
                                         stop=(pdi == ND - 1),
                                         tile_position=(0, 32 * (pbb % 4)))
                    pending = [(di, bb, ets[bb]) for bb in bbs]
                for pdi, pbb, pet in pending:
                    nc.tensor.matmul(psum2[pbb], cmods[pdi], pet[:],
                                     start=(pdi == 0), stop=(pdi == ND - 1),
                                     tile_position=(0, 32 * (pbb % 4)))
                    # out = -(psum2 + DREAL/2); alternate engines so the
                    # four epilogues drain in parallel
                    ot = opool.tile([C, 512], F32, name=f"ot_{pbb % 4}",
                                    tag="ot")
                    if pbb % 2 == 0:
                        nc.scalar.activation(ot[:], psum2[pbb], AF.Copy,
                                             bias=-float(DREAL) / 2.0,
                                             scale=-1.0)
                    else:
                        nc.vector.tensor_scalar(ot[:], psum2[pbb],
                                                float(DREAL) / 2.0, -1.0,
                                                op0=OP.add, op1=OP.mult)
                    nc.gpsimd.dma_start(
                        out[:, pbb * 512:(pbb + 1) * 512], ot[:])
    nc.compile()
    _NC_CACHE["nc"] = nc
    return nc


def _prep_in_maps(samples, bhv_matrix, centroids):
    samples = np.ascontiguousarray(samples, dtype=np.float32)
    bhv_matrix = np.ascontiguousarray(bhv_matrix, dtype=np.float32)
    centroids = np.ascontiguousarray(centroids, dtype=np.float32)
    xT = np.ascontiguousarray(samples.T)  # [F, B]
    in_maps = []
    for k in range(NCORES):
        lo_, hi_ = k * DREAL, (k + 1) * DREAL
        wTk = np.zeros((F, DP), dtype=np.float32)
        wTk[:, :DREAL] = bhv_matrix[lo_:hi_, :].T
        cTk = np.full((DP, C), 0.5, dtype=np.float32)
        cTk[:DREAL, :] = centroids[:, lo_:hi_].T
        in_maps.append({"xT": xT, "wT": wTk, "cT": cTk})
    return in_maps


def _run(samples, bhv_matrix, centroids, **spmd_kwargs):
    nc = _build_nc()
    in_maps = _prep_in_maps(samples, bhv_matrix, centroids)
    res = run_bass_kernel_spmd(nc, in_maps, core_ids=list(range(NCORES)),
                               **spmd_kwargs)
    acc = np.zeros((C, B), dtype=np.float32)
    for r in res.results:
        acc += r["out"]
    return np.ascontiguousarray(acc.T), res


def kernel(samples, bhv_matrix, centroids):
    out, _ = _run(samples, bhv_matrix, centroids)
    return out
